# revision 14
# baseline (speedup 1.0000x reference)
"""DPLSTMCell Trainium2 kernel — per-gate mixed precision (fp8 + fp16).

Data-parallel LSTM cell over 8 NeuronCores: batch dim of input/h_prev/c_prev
is sharded, the (small) weights are replicated.

Precision scheme (error budget rel<2e-2; measured rel_h≈1.62e-2):
  The four gate pre-activations have very different sensitivity to fp8
  quantization noise (h-error when ONLY that gate is fp8):
      i: 0.62e-2   f: 0.89e-2   o: 1.21e-2   g: 2.00e-2
  so gates i,f,o use fp8e4m3 DoubleRow matmuls (2 k-rows/cycle, 2x fp16
  throughput) while the tanh-gate g stays fp16.  Errors add in quadrature:
  sqrt(.62^2+.89^2+1.21^2) = 1.62e-2 < 2e-2.  PE row-work drops from 1024
  to 768 512-row matmuls per core (~221us -> ~140us roofline).

  Both operands are pre-scaled host-side by powers of two (x*32, W*4096,
  exact in fp16) so fp8 values avoid the subnormal range; the whole PSUM
  is then uniformly scaled by 2^17 and descaled for free by the ACT
  engine's `scale` immediate: sigmoid(2^-17 * (psum + bias*2^17)).

Layout: the gate dim is reordered into 2 slices of 2048 = [i|f|o|g]x512
for a contiguous 512-wide block of output h-dims, so every matmul chunk
is a full 512-wide PSUM bank write:
  psA [128,1536] (3 banks):  i,f,o  <- 3x8 fp8 DoubleRow matmuls (K=2048)
  psG [128, 512] (1 bank):   g      <- 16  fp16 matmuls
2-deep PSUM rotation pipelines chain b+1 on the PE while chain b's
epilogue drains on DVE/ACT/GpSimd.

Scheduling notes (hard-won):
  - DMA issue on a DGE queue costs ~0.6-0.8us per transfer, so small
    transfers are batched (c_prev in ONE upfront DMA, c|h outputs merged
    per slice) and split across two queues: W/bias/outputs on SP (sync),
    xh + c_prev on GpSimd.
  - The DVE queue is in-order; anything queued ahead of the next
    bias_add delays the PSUM free and stalls the PE.  bias_add is issued
    immediately after each chain; the i*g product goes to the (idle)
    GpSimd engine and the remaining tail ops only ever wait on work
    that completes well within a chain period.

Epilogue per (slice, b): DVE adds the pre-scaled bias out of PSUM (fp32),
ACT applies sigmoid/tanh with scale=2^-17 writing fp16 activations, and
the c/h tail runs in fp16 (DVE at 2x + one GpSimd op).  c_prev/h_t/c_t
move as fp16 (rel ~2e-4, negligible vs budget); the host upcasts.
"""

import numpy as np
import ml_dtypes

import concourse.bacc as bacc
import concourse.mybir as mybir
import concourse.tile as tile
from concourse.bass_utils import run_bass_kernel_spmd

AF = mybir.ActivationFunctionType
DR = mybir.MatmulPerfMode.DoubleRow
F8 = mybir.dt.float8e4
F16 = mybir.dt.float16
F32 = mybir.dt.float32

N_CORES = 8
B_TOTAL = 8192
IN_DIM = 1024
H_DIM = 1024
P = 128

SX = 32.0        # x pre-scale (power of two)
SW = 4096.0      # W pre-scale (power of two)
INV = 1.0 / (SX * SW)   # 2^-17, exact


def build_lstm_nc(b_loc=B_TOTAL // N_CORES, in_dim=IN_DIM, h_dim=H_DIM):
    ktot = in_dim + h_dim
    KT16 = ktot // P            # fp16 k-tiles (g gate)
    KT8 = ktot // (2 * P)       # fp8 DoubleRow k-super-tiles (i,f,o gates)
    G = 4 * h_dim               # total gate width
    NS = 2                      # slices, each [i|f|o|g] x DS
    SLW = G // NS               # slice width (2048)
    DS = h_dim // NS            # output-dim block per slice (512)
    W8C = 3 * DS                # fp8 cols per slice (1536: i,f,o)
    BT = b_loc // P             # batch tiles per core (8)

    nc = bacc.Bacc("TRN2", target_bir_lowering=False)
    # PE-ready host layouts; leading dim = SBUF partition (contraction k%128)
    xh16 = nc.dram_tensor("xh16", [P, BT, KT16, P], F16, kind="ExternalInput")
    xh8 = nc.dram_tensor("xh8", [P, BT, KT8, 2, P], F8, kind="ExternalInput")
    w16 = nc.dram_tensor("w16", [NS, P, KT16, DS], F16, kind="ExternalInput")
    w8 = nc.dram_tensor("w8", [NS, P, KT8, 2, W8C], F8, kind="ExternalInput")
    bias = nc.dram_tensor("bias", [P, G], F32, kind="ExternalInput")
    c_prev = nc.dram_tensor("c_prev", [P, BT, NS, DS], F16,
                            kind="ExternalInput")
    ch_out = nc.dram_tensor("ch_out", [b_loc, 2, h_dim], F16,
                            kind="ExternalOutput")

    with tile.TileContext(nc) as tc:
        with (
            tc.tile_pool(name="const", bufs=1) as const_pool,
            tc.tile_pool(name="xh", bufs=1) as xh_pool,
            tc.tile_pool(name="w8p", bufs=2) as w8_pool,
            tc.tile_pool(name="w16p", bufs=2) as w16_pool,
            tc.tile_pool(name="work", bufs=3) as work,
            tc.tile_pool(name="psA", bufs=2, space="PSUM") as psA_pool,
            tc.tile_pool(name="psG", bufs=2, space="PSUM") as psG_pool,
        ):
            xh16_sb = xh_pool.tile([P, BT, KT16, P], F16)
            xh8_sb = xh_pool.tile([P, BT, KT8, 2, P], F8)
            cp_sb = xh_pool.tile([P, BT, NS, DS], F16)
            bias_sb = const_pool.tile([P, G], F32)
            w8_tiles = {}
            w16_tiles = {}

            def alloc_w_slice(s):
                w8_tiles[s] = w8_pool.tile([P, KT8, 2, W8C], F8, name="w8s")
                w16_tiles[s] = w16_pool.tile([P, KT16, DS], F16, name="w16s")

            # ---- prologue DMA issue order matters: ~0.6-0.8us per issue ----
            # GpSimd queue: xh b0-3, c_prev (needed ~20us in), xh b4-7
            for b in range(4):
                nc.gpsimd.dma_start(xh8_sb[:, b], xh8[:, b])
                nc.gpsimd.dma_start(xh16_sb[:, b], xh16[:, b])
            nc.gpsimd.dma_start(cp_sb[:], c_prev[:, :, :, :])
            for b in range(4, BT):
                nc.gpsimd.dma_start(xh8_sb[:, b], xh8[:, b])
                nc.gpsimd.dma_start(xh16_sb[:, b], xh16[:, b])

            # Sync queue: slice-0 W split per-k and interleaved (w16 groups
            # among w8 tiles, in first-chain consumption order), then bias,
            # then slice-1 W as whole transfers.
            alloc_w_slice(0)
            w8_0, w16_0 = w8_tiles[0], w16_tiles[0]
            nc.sync.dma_start(w8_0[:, 0, :, :], w8[0, :, 0, :, :])
            nc.sync.dma_start(w8_0[:, 1, :, :], w8[0, :, 1, :, :])
            for j in range(3):
                nc.sync.dma_start(w8_0[:, 2 * j + 2:2 * j + 4, :, :],
                                  w8[0, :, 2 * j + 2:2 * j + 4, :, :])
                nc.sync.dma_start(w16_0[:, 4 * j:4 * j + 4, :],
                                  w16[0, :, 4 * j:4 * j + 4, :])
            nc.sync.dma_start(w16_0[:, 12:16, :], w16[0, :, 12:16, :])
            nc.sync.dma_start(bias_sb[:, 0:SLW], bias[:, 0:SLW])
            alloc_w_slice(1)
            nc.sync.dma_start(w8_tiles[1][:], w8[1, :, :, :, :])
            nc.sync.dma_start(w16_tiles[1][:], w16[1, :, :, :])
            nc.sync.dma_start(bias_sb[:, SLW:G], bias[:, SLW:G])

            # PE warmup: dummy matmuls on zeroed SBUF while the first W/xh
            # tiles stream in, so the PE p-state is at full clock when real
            # matmuls start.
            scratch = work.tile([P, 512], F16, name="scratch", bufs=1)
            nc.vector.memset(scratch[:], 0.0)
            zb = const_pool.tile([P, 1], F32)
            nc.vector.memset(zb[:], 0.0)
            ps_w = psG_pool.tile([P, DS], F32, name="psg")
            for i in range(8):
                nc.tensor.matmul(ps_w[:], scratch[:, 0:P], scratch[:],
                                 start=True, stop=True)

            def mm_fp8(ps, s, t, b):
                # i, f, o chunks: each a full 512-wide PSUM bank
                w8_s = w8_tiles[s]
                xsl = xh8_sb[:, b, t, :, :]
                st, sp = (t == 0), (t == KT8 - 1)
                for c in range(3):
                    nc.tensor.matmul(ps[:, c * DS:(c + 1) * DS], xsl,
                                     w8_s[:, t, :, c * DS:(c + 1) * DS],
                                     perf_mode=DR, start=st, stop=sp)

            def mm_fp16(ps, s, k, b):
                # g chunk (own PSUM bank)
                nc.tensor.matmul(ps[:], xh16_sb[:, b, k, :],
                                 w16_tiles[s][:, k, :],
                                 start=(k == 0), stop=(k == KT16 - 1))

            def bias_add(psa, psg, s):
                # gates_scaled = psum + bias*2^17 on the DVE; the ONLY psum
                # readers, so the PSUM tiles free right after.
                gates = work.tile([P, SLW], F32, name="gates", bufs=2)
                nc.vector.tensor_add(
                    gates[:, 0:W8C], psa[:],
                    bias_sb[:, s * SLW:s * SLW + W8C])
                nc.vector.tensor_add(
                    gates[:, W8C:SLW], psg[:],
                    bias_sb[:, s * SLW + W8C:(s + 1) * SLW])
                return gates

            def epilogue(gates, s, b):
                # slice layout: [ i | f | o | g ], each DS wide; ACT descales
                # by 2^-17 via its scale immediate, writes fp16 activations.
                act = work.tile([P, SLW], F16, name="act", bufs=2)
                nc.scalar.activation(act[:, 0:W8C], gates[:, 0:W8C],
                                     AF.Sigmoid, bias=zb[:], scale=INV)
                nc.scalar.activation(act[:, W8C:SLW],
                                     gates[:, W8C:SLW], AF.Tanh,
                                     bias=zb[:], scale=INV)
                # i*g on GpSimd so the DVE tail can't delay the next
                # bias_add by more than its own (short) ops.
                ig = work.tile([P, DS], F16, name="ig")
                nc.gpsimd.tensor_mul(ig[:], act[:, 0:DS],
                                     act[:, 3 * DS:4 * DS])
                chnew = work.tile([P, 2, DS], F16, name="chnew")
                cnew = chnew[:, 0, :]
                nc.vector.tensor_mul(cnew, act[:, DS:2 * DS],
                                     cp_sb[:, b, s, :])
                nc.vector.tensor_add(cnew, cnew, ig[:])
                tct = work.tile([P, DS], F16, name="tct")
                nc.scalar.activation(tct[:], cnew, AF.Tanh, bias=zb[:])
                nc.vector.tensor_mul(chnew[:, 1, :], act[:, 2 * DS:3 * DS],
                                     tct[:])
                nc.sync.dma_start(
                    ch_out[b * P:(b + 1) * P, :, s * DS:(s + 1) * DS],
                    chnew[:, :, :])

            # per-b chains; 2-deep PSUM rotation pipelines chain b+1 on the
            # PE while chain b's epilogue drains on DVE/ACT/GpSimd.
            for s in range(NS):
                for b in range(BT):
                    psa = psA_pool.tile([P, W8C], F32, name="psa")
                    psg = psG_pool.tile([P, DS], F32, name="psg")
                    for t in range(KT8):
                        mm_fp8(psa, s, t, b)
                    for k in range(KT16):
                        mm_fp16(psg, s, k, b)
                    epilogue(bias_add(psa, psg, s), s, b)

    nc.compile()
    return nc


def prep_inputs(input, h_prev, c_prev, W_ih, b_ih, W_hh, b_hh,
                n_cores=N_CORES):
    """Host-side shard + quantize + layout prep. Per-core input maps."""
    input = np.asarray(input, np.float32)
    h_prev = np.asarray(h_prev, np.float32)
    c_prev = np.asarray(c_prev, np.float32)
    W_ih = np.asarray(W_ih, np.float32)
    W_hh = np.asarray(W_hh, np.float32)
    b_ih = np.asarray(b_ih, np.float32)
    b_hh = np.asarray(b_hh, np.float32)

    b_total, in_dim = input.shape
    h_dim = h_prev.shape[1]
    ktot = in_dim + h_dim
    b_loc = b_total // n_cores
    G = 4 * h_dim
    NS = 2
    DS = h_dim // NS
    W8C = 3 * DS
    SLW = G // NS
    BT = b_loc // 128
    KT16 = ktot // 128
    KT8 = ktot // 256

    def q8(x):
        return np.clip(x, -240, 240).astype(ml_dtypes.float8_e4m3)

    # column reorder: per slice s the layout is [i | f | o | g] for output
    # dims [s*DS, (s+1)*DS)
    arr = np.arange(G).reshape(4, NS, DS)       # [gate, s, r]
    idx = arr[[0, 1, 3, 2]].transpose(1, 0, 2).reshape(-1)

    Ws = np.concatenate([W_ih, W_hh], axis=1)[idx, :] * SW   # [G, ktot]
    # fp8 blocks (i,f,o = first 1536 cols of each slice) in DoubleRow layout
    w8_host = np.empty((NS, 128, KT8, 2, W8C), ml_dtypes.float8_e4m3)
    w16_host = np.empty((NS, 128, KT16, DS), np.float16)
    for s in range(NS):
        blk8 = q8(Ws[s * SLW:s * SLW + W8C, :]).T            # [ktot, 1536]
        w8_host[s] = blk8.reshape(KT8, 2, 128, W8C).transpose(2, 0, 1, 3)
        blk16 = Ws[s * SLW + W8C:(s + 1) * SLW, :].T.astype(np.float16)
        w16_host[s] = blk16.reshape(KT16, 128, DS).transpose(1, 0, 2)

    bias_row = ((b_ih + b_hh)[idx] * (SX * SW)).astype(np.float32)
    bias = np.ascontiguousarray(np.broadcast_to(bias_row, (128, G)))

    xh = np.concatenate([input, h_prev], axis=1) * SX        # [B, ktot]
    x8 = q8(xh)
    x16 = xh.astype(np.float16)
    c16 = c_prev.astype(np.float16)

    in_maps = []
    for c in range(n_cores):
        rows = slice(c * b_loc, (c + 1) * b_loc)
        xc8 = x8[rows].T                                     # [ktot, b_loc]
        xc16 = x16[rows].T
        # [p, b, t, s, m] = x[t*256 + s*128 + p, b*128 + m]
        xh8_h = xc8.reshape(KT8, 2, 128, BT, 128).transpose(2, 3, 0, 1, 4)
        xh16_h = xc16.reshape(KT16, 128, BT, 128).transpose(1, 2, 0, 3)
        # c_prev: [p, b, s, r] = c[b*128 + p, s*DS + r]
        cp_h = c16[rows].reshape(BT, 128, NS, DS).transpose(1, 0, 2, 3)
        in_maps.append({
            "xh8": np.ascontiguousarray(xh8_h),
            "xh16": np.ascontiguousarray(xh16_h),
            "w8": w8_host,
            "w16": w16_host,
            "bias": bias,
            "c_prev": np.ascontiguousarray(cp_h),
        })
    return in_maps


def run_lstm(inputs, trace=False, **spmd_kwargs):
    """Builds + runs the kernel on all 8 cores. Returns (h_t, c_t), results."""
    in_maps = prep_inputs(**inputs)
    nc = build_lstm_nc()
    res = run_bass_kernel_spmd(nc, in_maps, core_ids=list(range(N_CORES)),
                               trace=trace, **spmd_kwargs)
    ch = np.concatenate([r["ch_out"] for r in res.results], axis=0)
    c_t = ch[:, 0, :].astype(np.float32)
    h_t = ch[:, 1, :].astype(np.float32)
    return (h_t, c_t), res


def kernel(input, h_prev, c_prev, W_ih, b_ih, W_hh, b_hh):
    (h_t, c_t), _ = run_lstm(dict(
        input=input, h_prev=h_prev, c_prev=c_prev,
        W_ih=W_ih, b_ih=b_ih, W_hh=W_hh, b_hh=b_hh))
    return (h_t, c_t)


# revision 15
# speedup vs baseline: 1.0062x; 1.0062x over previous
"""DPLSTMCell Trainium2 kernel — per-gate mixed precision (fp8 + fp16).

Data-parallel LSTM cell over 8 NeuronCores: batch dim of input/h_prev/c_prev
is sharded, the (small) weights are replicated.

Precision scheme (error budget rel<2e-2; measured rel_h≈1.62e-2):
  The four gate pre-activations have very different sensitivity to fp8
  quantization noise (h-error when ONLY that gate is fp8):
      i: 0.62e-2   f: 0.89e-2   o: 1.21e-2   g: 2.00e-2
  so gates i,f,o use fp8e4m3 DoubleRow matmuls (2 k-rows/cycle, 2x fp16
  throughput) while the tanh-gate g stays fp16.  Errors add in quadrature:
  sqrt(.62^2+.89^2+1.21^2) = 1.62e-2 < 2e-2.  PE row-work drops from 1024
  to 768 512-row matmuls per core (~221us -> ~140us roofline).

  Both operands are pre-scaled host-side by powers of two (x*32, W*4096,
  exact in fp16) so fp8 values avoid the subnormal range; the whole PSUM
  is then uniformly scaled by 2^17 and descaled for free by the ACT
  engine's `scale` immediate: sigmoid(2^-17 * (psum + bias*2^17)).

Layout: the gate dim is reordered into 2 slices of 2048 = [i|f|o|g]x512
for a contiguous 512-wide block of output h-dims, so every matmul chunk
is a full 512-wide PSUM bank write:
  psA [128,1536] (3 banks):  i,f,o  <- 3x8 fp8 DoubleRow matmuls (K=2048)
  psG [128, 512] (1 bank):   g      <- 16  fp16 matmuls
2-deep PSUM rotation pipelines chain b+1 on the PE while chain b's
epilogue drains on DVE/ACT/GpSimd.

Scheduling notes (hard-won):
  - DMA issue on a DGE queue costs ~0.6-0.8us per transfer, so small
    transfers are batched (c_prev in ONE upfront DMA, c|h outputs merged
    per slice) and split across two queues: W/bias/outputs on SP (sync),
    xh + c_prev on GpSimd.
  - The DVE queue is in-order; anything queued ahead of the next
    bias_add delays the PSUM free and stalls the PE.  bias_add is issued
    immediately after each chain; the i*g product goes to the (idle)
    GpSimd engine and the remaining tail ops only ever wait on work
    that completes well within a chain period.

Epilogue per (slice, b): DVE adds the pre-scaled bias out of PSUM (fp32),
ACT applies sigmoid/tanh with scale=2^-17 writing fp16 activations, and
the c/h tail runs in fp16 (DVE at 2x + one GpSimd op).  c_prev/h_t/c_t
move as fp16 (rel ~2e-4, negligible vs budget); the host upcasts.
"""

import numpy as np
import ml_dtypes

import concourse.bacc as bacc
import concourse.mybir as mybir
import concourse.tile as tile
from concourse.bass_utils import run_bass_kernel_spmd

AF = mybir.ActivationFunctionType
DR = mybir.MatmulPerfMode.DoubleRow
F8 = mybir.dt.float8e4
F16 = mybir.dt.float16
F32 = mybir.dt.float32

N_CORES = 8
B_TOTAL = 8192
IN_DIM = 1024
H_DIM = 1024
P = 128

SX = 32.0        # x pre-scale (power of two)
SW = 4096.0      # W pre-scale (power of two)
INV = 1.0 / (SX * SW)   # 2^-17, exact


def build_lstm_nc(b_loc=B_TOTAL // N_CORES, in_dim=IN_DIM, h_dim=H_DIM):
    ktot = in_dim + h_dim
    KT16 = ktot // P            # fp16 k-tiles (g gate)
    KT8 = ktot // (2 * P)       # fp8 DoubleRow k-super-tiles (i,f,o gates)
    G = 4 * h_dim               # total gate width
    NS = 2                      # slices, each [i|f|o|g] x DS
    SLW = G // NS               # slice width (2048)
    DS = h_dim // NS            # output-dim block per slice (512)
    W8C = 3 * DS                # fp8 cols per slice (1536: i,f,o)
    BT = b_loc // P             # batch tiles per core (8)

    nc = bacc.Bacc("TRN2", target_bir_lowering=False)
    # PE-ready host layouts; leading dim = SBUF partition (contraction k%128)
    xh16 = nc.dram_tensor("xh16", [P, BT, KT16, P], F16, kind="ExternalInput")
    xh8 = nc.dram_tensor("xh8", [P, BT, KT8, 2, P], F8, kind="ExternalInput")
    w16 = nc.dram_tensor("w16", [NS, P, KT16, DS], F16, kind="ExternalInput")
    w8 = nc.dram_tensor("w8", [NS, P, KT8, 3, 2, DS], F8,
                        kind="ExternalInput")
    bias = nc.dram_tensor("bias", [P, G], F32, kind="ExternalInput")
    c_prev = nc.dram_tensor("c_prev", [P, BT, NS, DS], F16,
                            kind="ExternalInput")
    ch_out = nc.dram_tensor("ch_out", [b_loc, 2, h_dim], F16,
                            kind="ExternalOutput")

    with tile.TileContext(nc) as tc:
        with (
            tc.tile_pool(name="const", bufs=1) as const_pool,
            tc.tile_pool(name="xh", bufs=1) as xh_pool,
            tc.tile_pool(name="w8p", bufs=2) as w8_pool,
            tc.tile_pool(name="w16p", bufs=2) as w16_pool,
            tc.tile_pool(name="work", bufs=3) as work,
            tc.tile_pool(name="psA", bufs=2, space="PSUM") as psA_pool,
            tc.tile_pool(name="psG", bufs=2, space="PSUM") as psG_pool,
        ):
            xh16_sb = xh_pool.tile([P, BT, KT16, P], F16)
            xh8_sb = xh_pool.tile([P, BT, KT8, 2, P], F8)
            cp_sb = xh_pool.tile([P, BT, NS, DS], F16)
            bias_sb = const_pool.tile([P, G], F32)
            w8_tiles = {}
            w16_tiles = {}

            def alloc_w_slice(s):
                w8_tiles[s] = w8_pool.tile([P, KT8, 3, 2, DS], F8,
                                           name="w8s")
                w16_tiles[s] = w16_pool.tile([P, KT16, DS], F16, name="w16s")

            # ---- prologue DMA issue order matters: ~0.6-0.8us per issue ----
            # GpSimd queue: xh b0-3, c_prev (needed ~20us in), xh b4-7
            for b in range(4):
                nc.gpsimd.dma_start(xh8_sb[:, b], xh8[:, b])
                nc.gpsimd.dma_start(xh16_sb[:, b], xh16[:, b])
            nc.gpsimd.dma_start(cp_sb[:], c_prev[:, :, :, :])
            for b in range(4, BT):
                nc.gpsimd.dma_start(xh8_sb[:, b], xh8[:, b])
                nc.gpsimd.dma_start(xh16_sb[:, b], xh16[:, b])

            # Sync queue: slice-0 W split per-k and interleaved (w16 groups
            # among w8 tiles, in first-chain consumption order), then bias,
            # then slice-1 W as whole transfers.
            alloc_w_slice(0)
            w8_0, w16_0 = w8_tiles[0], w16_tiles[0]
            nc.sync.dma_start(w8_0[:, 0], w8[0, :, 0])
            nc.sync.dma_start(w8_0[:, 1], w8[0, :, 1])
            for j in range(3):
                nc.sync.dma_start(w8_0[:, 2 * j + 2:2 * j + 4],
                                  w8[0, :, 2 * j + 2:2 * j + 4])
                nc.sync.dma_start(w16_0[:, 4 * j:4 * j + 4, :],
                                  w16[0, :, 4 * j:4 * j + 4, :])
            nc.sync.dma_start(w16_0[:, 12:16, :], w16[0, :, 12:16, :])
            nc.sync.dma_start(bias_sb[:, 0:SLW], bias[:, 0:SLW])
            alloc_w_slice(1)
            nc.sync.dma_start(w8_tiles[1][:], w8[1])
            nc.sync.dma_start(w16_tiles[1][:], w16[1, :, :, :])
            nc.sync.dma_start(bias_sb[:, SLW:G], bias[:, SLW:G])

            # PE warmup: dummy matmuls on zeroed SBUF while the first W/xh
            # tiles stream in, so the PE p-state is at full clock when real
            # matmuls start.
            scratch = work.tile([P, 512], F16, name="scratch", bufs=1)
            nc.vector.memset(scratch[:], 0.0)
            zb = const_pool.tile([P, 1], F32)
            nc.vector.memset(zb[:], 0.0)
            ps_w = psG_pool.tile([P, DS], F32, name="psg")
            for i in range(8):
                nc.tensor.matmul(ps_w[:], scratch[:, 0:P], scratch[:],
                                 start=True, stop=True)

            def mm_fp8(ps, s, t, b):
                # i, f, o chunks: each a full 512-wide PSUM bank
                w8_s = w8_tiles[s]
                xsl = xh8_sb[:, b, t, :, :]
                st, sp = (t == 0), (t == KT8 - 1)
                for c in range(3):
                    nc.tensor.matmul(ps[:, c * DS:(c + 1) * DS], xsl,
                                     w8_s[:, t, c, :, :],
                                     perf_mode=DR, start=st, stop=sp)

            def mm_fp16(ps, s, k, b):
                # g chunk (own PSUM bank)
                nc.tensor.matmul(ps[:], xh16_sb[:, b, k, :],
                                 w16_tiles[s][:, k, :],
                                 start=(k == 0), stop=(k == KT16 - 1))

            def bias_add(psa, psg, s):
                # gates_scaled = psum + bias*2^17 on the DVE; the ONLY psum
                # readers, so the PSUM tiles free right after.
                gates = work.tile([P, SLW], F32, name="gates", bufs=2)
                nc.vector.tensor_add(
                    gates[:, 0:W8C], psa[:],
                    bias_sb[:, s * SLW:s * SLW + W8C])
                nc.vector.tensor_add(
                    gates[:, W8C:SLW], psg[:],
                    bias_sb[:, s * SLW + W8C:(s + 1) * SLW])
                return gates

            def epilogue(gates, s, b):
                # slice layout: [ i | f | o | g ], each DS wide; ACT descales
                # by 2^-17 via its scale immediate, writes fp16 activations.
                act = work.tile([P, SLW], F16, name="act", bufs=2)
                nc.scalar.activation(act[:, 0:W8C], gates[:, 0:W8C],
                                     AF.Sigmoid, bias=zb[:], scale=INV)
                nc.scalar.activation(act[:, W8C:SLW],
                                     gates[:, W8C:SLW], AF.Tanh,
                                     bias=zb[:], scale=INV)
                ig = work.tile([P, DS], F16, name="ig")
                nc.vector.tensor_mul(ig[:], act[:, 0:DS],
                                     act[:, 3 * DS:4 * DS])
                chnew = work.tile([P, 2, DS], F16, name="chnew")
                cnew = chnew[:, 0, :]
                nc.vector.tensor_mul(cnew, act[:, DS:2 * DS],
                                     cp_sb[:, b, s, :])
                nc.vector.tensor_add(cnew, cnew, ig[:])
                tct = work.tile([P, DS], F16, name="tct")
                nc.scalar.activation(tct[:], cnew, AF.Tanh, bias=zb[:])
                nc.vector.tensor_mul(chnew[:, 1, :], act[:, 2 * DS:3 * DS],
                                     tct[:])
                nc.sync.dma_start(
                    ch_out[b * P:(b + 1) * P, :, s * DS:(s + 1) * DS],
                    chnew[:, :, :])

            # per-b chains; 2-deep PSUM rotation pipelines chain b+1 on the
            # PE while chain b's epilogue drains on DVE/ACT/GpSimd.
            for s in range(NS):
                for b in range(BT):
                    psa = psA_pool.tile([P, W8C], F32, name="psa")
                    psg = psG_pool.tile([P, DS], F32, name="psg")
                    for t in range(KT8):
                        mm_fp8(psa, s, t, b)
                    for k in range(KT16):
                        mm_fp16(psg, s, k, b)
                    epilogue(bias_add(psa, psg, s), s, b)

    nc.compile()
    return nc


def prep_inputs(input, h_prev, c_prev, W_ih, b_ih, W_hh, b_hh,
                n_cores=N_CORES):
    """Host-side shard + quantize + layout prep. Per-core input maps."""
    input = np.asarray(input, np.float32)
    h_prev = np.asarray(h_prev, np.float32)
    c_prev = np.asarray(c_prev, np.float32)
    W_ih = np.asarray(W_ih, np.float32)
    W_hh = np.asarray(W_hh, np.float32)
    b_ih = np.asarray(b_ih, np.float32)
    b_hh = np.asarray(b_hh, np.float32)

    b_total, in_dim = input.shape
    h_dim = h_prev.shape[1]
    ktot = in_dim + h_dim
    b_loc = b_total // n_cores
    G = 4 * h_dim
    NS = 2
    DS = h_dim // NS
    W8C = 3 * DS
    SLW = G // NS
    BT = b_loc // 128
    KT16 = ktot // 128
    KT8 = ktot // 256

    def q8(x):
        return np.clip(x, -240, 240).astype(ml_dtypes.float8_e4m3)

    # column reorder: per slice s the layout is [i | f | o | g] for output
    # dims [s*DS, (s+1)*DS)
    arr = np.arange(G).reshape(4, NS, DS)       # [gate, s, r]
    idx = arr[[0, 1, 3, 2]].transpose(1, 0, 2).reshape(-1)

    Ws = np.concatenate([W_ih, W_hh], axis=1)[idx, :] * SW   # [G, ktot]
    # fp8 blocks (i,f,o = first 1536 cols of each slice) in DoubleRow layout
    w8_host = np.empty((NS, 128, KT8, 3, 2, DS), ml_dtypes.float8_e4m3)
    w16_host = np.empty((NS, 128, KT16, DS), np.float16)
    for s in range(NS):
        blk8 = q8(Ws[s * SLW:s * SLW + W8C, :]).T            # [ktot, 1536]
        w8_host[s] = (blk8.reshape(KT8, 2, 128, 3, DS)
                      .transpose(2, 0, 3, 1, 4))
        blk16 = Ws[s * SLW + W8C:(s + 1) * SLW, :].T.astype(np.float16)
        w16_host[s] = blk16.reshape(KT16, 128, DS).transpose(1, 0, 2)

    bias_row = ((b_ih + b_hh)[idx] * (SX * SW)).astype(np.float32)
    bias = np.ascontiguousarray(np.broadcast_to(bias_row, (128, G)))

    xh = np.concatenate([input, h_prev], axis=1) * SX        # [B, ktot]
    x8 = q8(xh)
    x16 = xh.astype(np.float16)
    c16 = c_prev.astype(np.float16)

    in_maps = []
    for c in range(n_cores):
        rows = slice(c * b_loc, (c + 1) * b_loc)
        xc8 = x8[rows].T                                     # [ktot, b_loc]
        xc16 = x16[rows].T
        # [p, b, t, s, m] = x[t*256 + s*128 + p, b*128 + m]
        xh8_h = xc8.reshape(KT8, 2, 128, BT, 128).transpose(2, 3, 0, 1, 4)
        xh16_h = xc16.reshape(KT16, 128, BT, 128).transpose(1, 2, 0, 3)
        # c_prev: [p, b, s, r] = c[b*128 + p, s*DS + r]
        cp_h = c16[rows].reshape(BT, 128, NS, DS).transpose(1, 0, 2, 3)
        in_maps.append({
            "xh8": np.ascontiguousarray(xh8_h),
            "xh16": np.ascontiguousarray(xh16_h),
            "w8": w8_host,
            "w16": w16_host,
            "bias": bias,
            "c_prev": np.ascontiguousarray(cp_h),
        })
    return in_maps


def run_lstm(inputs, trace=False, **spmd_kwargs):
    """Builds + runs the kernel on all 8 cores. Returns (h_t, c_t), results."""
    in_maps = prep_inputs(**inputs)
    nc = build_lstm_nc()
    res = run_bass_kernel_spmd(nc, in_maps, core_ids=list(range(N_CORES)),
                               trace=trace, **spmd_kwargs)
    ch = np.concatenate([r["ch_out"] for r in res.results], axis=0)
    c_t = ch[:, 0, :].astype(np.float32)
    h_t = ch[:, 1, :].astype(np.float32)
    return (h_t, c_t), res


def kernel(input, h_prev, c_prev, W_ih, b_ih, W_hh, b_hh):
    (h_t, c_t), _ = run_lstm(dict(
        input=input, h_prev=h_prev, c_prev=c_prev,
        W_ih=W_ih, b_ih=b_ih, W_hh=W_hh, b_hh=b_hh))
    return (h_t, c_t)


# revision 18
# speedup vs baseline: 1.0580x; 1.0515x over previous
"""DPLSTMCell Trainium2 kernel — per-gate mixed precision (fp8 + fp16).

Data-parallel LSTM cell over 8 NeuronCores: batch dim of input/h_prev/c_prev
is sharded, the (small) weights are replicated.

Precision scheme (error budget rel<2e-2; measured rel_h≈1.6e-2):
  The four gate pre-activations have very different sensitivity to fp8
  quantization noise (h-error when ONLY that gate is fp8):
      i: 0.62e-2   f: 0.89e-2   o: 1.21e-2   g: 2.00e-2
  so gates i,f,o use fp8e4m3 DoubleRow matmuls (2 k-rows/cycle, 2x fp16
  throughput) while the tanh-gate g stays fp16.  Errors add in quadrature:
  sqrt(.62^2+.89^2+1.21^2) = 1.62e-2 < 2e-2.  PE work drops from 1024 to
  768 x 512-row-equivalents per core (~138us vs ~221us roofline).

  Both operands are pre-scaled host-side by powers of two (x*32, W*4096,
  exact in fp16) so fp8 values avoid the subnormal range; the whole PSUM
  is then uniformly scaled by 2^17 and descaled for free by the ACT
  engine's `scale` immediate: sigmoid(2^-17 * (psum + bias*2^17)).

Host-side prep (not part of HW exec time):
  - columns of W reordered so each 1024-wide "quarter" holds a full
    [i|f|o|g] set (256 each) for a contiguous slice of output dims; fp8
    blocks (i,f,o = 768 cols) and fp16 block (g = 256 cols) are packed
    into PE-ready DoubleRow / k-tile layouts with the contraction dim on
    SBUF partitions.
  - xh = concat(input, h_prev): quantized once to fp8 (DoubleRow pair
    layout) and once to fp16, batch-tile-major so per-b-tile DMAs land
    in compute order.
Device kernel (per core, B_loc = B/8 = 1024):
  per (quarter q, batch-tile b): PSUM tile [128,1024] accumulates
    cols 0:512   (i|f): 8 fp8 DoubleRow matmuls over K=2048
    cols 512:768 (o):   8 fp8 DoubleRow matmuls
    cols 768:1024(g):  16 fp16 matmuls
  DVE adds the (pre-scaled) bias, ACT applies sigmoid/tanh with
  scale=2^-17, then the usual c/h elementwise tail on DVE + ACT.
"""

import numpy as np
import ml_dtypes

import concourse.bacc as bacc
import concourse.mybir as mybir
import concourse.tile as tile
from concourse.bass_utils import run_bass_kernel_spmd

AF = mybir.ActivationFunctionType
DR = mybir.MatmulPerfMode.DoubleRow
F8 = mybir.dt.float8e4
F16 = mybir.dt.float16
F32 = mybir.dt.float32

N_CORES = 8
B_TOTAL = 8192
IN_DIM = 1024
H_DIM = 1024
P = 128

SX = 32.0        # x pre-scale (power of two)
SW = 4096.0      # W pre-scale (power of two)
INV = 1.0 / (SX * SW)   # 2^-17, exact


def build_lstm_nc(b_loc=B_TOTAL // N_CORES, in_dim=IN_DIM, h_dim=H_DIM):
    ktot = in_dim + h_dim
    KT16 = ktot // P            # fp16 k-tiles (g gate)
    KT8 = ktot // (2 * P)       # fp8 DoubleRow k-super-tiles (i,f,o gates)
    G = 4 * h_dim               # total gate width
    NQ = 4                      # quarters, each [i|f|o|g] x DS
    QW = G // NQ                # quarter width (1024)
    DS = h_dim // NQ            # output-dim slice per quarter (256)
    W8C = 3 * DS                # fp8 cols per quarter (768: i,f,o)
    BT = b_loc // P             # batch tiles per core (8)
    GRP = min(4, BT)            # batch tiles in flight for quarter 0

    nc = bacc.Bacc("TRN2", target_bir_lowering=False)
    # PE-ready host layouts; leading dim = SBUF partition (contraction k%128)
    xh16 = nc.dram_tensor("xh16", [P, BT, KT16, P], F16, kind="ExternalInput")
    xh8 = nc.dram_tensor("xh8", [P, BT, KT8, 2, P], F8, kind="ExternalInput")
    w16 = nc.dram_tensor("w16", [NQ, P, KT16, DS], F16, kind="ExternalInput")
    w8 = nc.dram_tensor("w8", [NQ, P, KT8, 2, W8C], F8, kind="ExternalInput")
    bias = nc.dram_tensor("bias", [P, G], F32, kind="ExternalInput")
    c_prev = nc.dram_tensor("c_prev", [P, BT, NQ, DS], F16,
                            kind="ExternalInput")
    ch_out = nc.dram_tensor("ch_out", [b_loc, 2, h_dim], F16,
                            kind="ExternalOutput")

    with tile.TileContext(nc) as tc:
        with (
            tc.tile_pool(name="const", bufs=1) as const_pool,
            tc.tile_pool(name="xh", bufs=1) as xh_pool,
            tc.tile_pool(name="w8p", bufs=2) as w8_pool,
            tc.tile_pool(name="w16p", bufs=2) as w16_pool,
            tc.tile_pool(name="work", bufs=3) as work,
            tc.tile_pool(name="psum", bufs=4, space="PSUM") as psum_pool,
        ):
            xh16_sb = xh_pool.tile([P, BT, KT16, P], F16)
            xh8_sb = xh_pool.tile([P, BT, KT8, 2, P], F8)
            cp_sb = xh_pool.tile([P, BT, NQ, DS], F16)
            bias_sb = const_pool.tile([P, G], F32)
            w8_tiles = {}
            w16_tiles = {}

            def load_w_quarter(q, split=False):
                w8_q = w8_pool.tile([P, KT8, 2, W8C], F8, name="w8q")
                w16_q = w16_pool.tile([P, KT16, DS], F16, name="w16q")
                w8_tiles[q] = w8_q
                w16_tiles[q] = w16_q
                if not split:
                    nc.sync.dma_start(w8_q[:], w8[q, :, :, :, :])
                    nc.sync.dma_start(w16_q[:], w16[q, :, :, :])
                    nc.sync.dma_start(bias_sb[:, q * QW:(q + 1) * QW],
                                      bias[:, q * QW:(q + 1) * QW])
                    return
                # quarter 0: split per-k so the first batch group's
                # chains unblock as early as possible (xh streams in
                # parallel on the gpsimd queue).
                for t in range(KT8):
                    nc.sync.dma_start(w8_q[:, t, :, :], w8[q, :, t, :, :])
                for k in range(0, KT16, 4):
                    nc.sync.dma_start(w16_q[:, k:k + 4, :],
                                      w16[q, :, k:k + 4, :])
                nc.sync.dma_start(bias_sb[:, q * QW:(q + 1) * QW],
                                  bias[:, q * QW:(q + 1) * QW])

            # xh + c_prev on the gpsimd DMA queue, in compute order
            for b in range(GRP):
                nc.gpsimd.dma_start(xh8_sb[:, b], xh8[:, b])
            for b in range(GRP):
                nc.gpsimd.dma_start(xh16_sb[:, b], xh16[:, b])
            nc.gpsimd.dma_start(cp_sb[:], c_prev[:, :, :, :])
            for b in range(GRP, BT):
                nc.gpsimd.dma_start(xh8_sb[:, b], xh8[:, b])
                nc.gpsimd.dma_start(xh16_sb[:, b], xh16[:, b])

            load_w_quarter(0, split=True)

            # PE warmup: dummy matmuls on zeroed SBUF while the first W/xh
            # tiles stream in, so the PE p-state is at full clock when real
            # matmuls start.
            scratch = work.tile([P, 512], F16, name="scratch", bufs=1)
            nc.vector.memset(scratch[:], 0.0)
            zb = const_pool.tile([P, 1], F32)
            nc.vector.memset(zb[:], 0.0)
            ps_w = psum_pool.tile([P, QW], F32, name="ps")
            for i in range(8):
                nc.tensor.matmul(
                    ps_w[:, (i % 2) * 512:(i % 2) * 512 + 512],
                    scratch[:, 0:P], scratch[:],
                    start=True, stop=True)

            def mm_fp8(ps, q, t, b):
                # i|f chunk (cols 0:512, psum bank A) and o chunk (512:768)
                w8_q = w8_tiles[q]
                xsl = xh8_sb[:, b, t, :, :]
                nc.tensor.matmul(ps[:, 0:512], xsl, w8_q[:, t, :, 0:512],
                                 perf_mode=DR,
                                 start=(t == 0), stop=(t == KT8 - 1))
                nc.tensor.matmul(ps[:, 512:W8C], xsl, w8_q[:, t, :, 512:W8C],
                                 perf_mode=DR,
                                 start=(t == 0), stop=(t == KT8 - 1))

            def mm_fp16(ps, q, k, b):
                # g chunk (cols 768:1024, psum bank B)
                nc.tensor.matmul(ps[:, W8C:QW],
                                 xh16_sb[:, b, k, :],
                                 w16_tiles[q][:, k, :],
                                 start=(k == 0), stop=(k == KT16 - 1))

            def bias_add(ps, q):
                # gates_scaled = psum + bias*2^17 on the DVE; the ONLY psum
                # reader, so the PSUM slot frees right after it.
                gates = work.tile([P, QW], F32, name="gates", bufs=8)
                nc.vector.tensor_add(
                    gates[:], ps[:], bias_sb[:, q * QW:(q + 1) * QW])
                return gates

            def epilogue(gates, q, b):
                # quarter layout: [ i | f | o | g ], each DS wide; ACT
                # descales by 2^-17 via its scale immediate, writes fp16
                # activations so the DVE tail runs in 2x fp16 mode.
                act = work.tile([P, QW], F16, name="act", bufs=8)
                nc.scalar.activation(act[:, 0:3 * DS], gates[:, 0:3 * DS],
                                     AF.Sigmoid, bias=zb[:], scale=INV)
                nc.scalar.activation(act[:, 3 * DS:4 * DS],
                                     gates[:, 3 * DS:4 * DS], AF.Tanh,
                                     bias=zb[:], scale=INV)

                ig = work.tile([P, DS], F16, name="ig")
                nc.vector.tensor_mul(ig[:], act[:, 0:DS],
                                     act[:, 3 * DS:4 * DS])
                chnew = work.tile([P, 2, DS], F16, name="chnew")
                cnew = chnew[:, 0, :]
                nc.vector.tensor_mul(cnew, act[:, DS:2 * DS],
                                     cp_sb[:, b, q, :])
                nc.vector.tensor_add(cnew, cnew, ig[:])
                tct = work.tile([P, DS], F16, name="tct")
                nc.scalar.activation(tct[:], cnew, AF.Tanh, bias=zb[:])
                nc.vector.tensor_mul(chnew[:, 1, :], act[:, 2 * DS:3 * DS],
                                     tct[:])
                nc.sync.dma_start(
                    ch_out[b * P:(b + 1) * P, :, q * DS:(q + 1) * DS],
                    chnew[:, :, :])

            # ---- quarter 0: k-outer over GRP-wide batch groups so matmuls
            # start while W/xh stream in ----
            for g0 in range(0, BT, GRP):
                nb = min(GRP, BT - g0)
                pss = [psum_pool.tile([P, QW], F32, name="ps")
                       for _ in range(nb)]
                for t in range(KT8):
                    for bi, ps in enumerate(pss):
                        mm_fp8(ps, 0, t, g0 + bi)
                for k in range(KT16):
                    for bi, ps in enumerate(pss):
                        mm_fp16(ps, 0, k, g0 + bi)
                gts = [bias_add(ps, 0) for ps in pss]
                for bi, gates in enumerate(gts):
                    epilogue(gates, 0, g0 + bi)

            # ---- quarters 1..3: prefetched, dense per-b chains ----
            for q in range(1, NQ):
                load_w_quarter(q)
                for b in range(BT):
                    ps = psum_pool.tile([P, QW], F32, name="ps")
                    for t in range(KT8):
                        mm_fp8(ps, q, t, b)
                    for k in range(KT16):
                        mm_fp16(ps, q, k, b)
                    epilogue(bias_add(ps, q), q, b)

    nc.compile()
    return nc


def prep_inputs(input, h_prev, c_prev, W_ih, b_ih, W_hh, b_hh,
                n_cores=N_CORES):
    """Host-side shard + quantize + layout prep. Per-core input maps."""
    input = np.asarray(input, np.float32)
    h_prev = np.asarray(h_prev, np.float32)
    c_prev = np.asarray(c_prev, np.float32)
    W_ih = np.asarray(W_ih, np.float32)
    W_hh = np.asarray(W_hh, np.float32)
    b_ih = np.asarray(b_ih, np.float32)
    b_hh = np.asarray(b_hh, np.float32)

    b_total, in_dim = input.shape
    h_dim = h_prev.shape[1]
    ktot = in_dim + h_dim
    b_loc = b_total // n_cores
    G = 4 * h_dim
    NQ = 4
    DS = h_dim // NQ
    W8C = 3 * DS
    BT = b_loc // 128
    KT16 = ktot // 128
    KT8 = ktot // 256
    # (c_prev is pre-tiled to [128, BT, NQ, DS] fp16 below)

    def q8(x):
        return np.clip(x, -240, 240).astype(ml_dtypes.float8_e4m3)

    # column reorder: per quarter q the layout is [i | f | o | g] for output
    # dims [q*DS, (q+1)*DS)
    arr = np.arange(G).reshape(4, NQ, DS)       # [gate, q, r]
    idx = arr[[0, 1, 3, 2]].transpose(1, 0, 2).reshape(-1)

    W_cat = np.concatenate([W_ih, W_hh], axis=1)[idx, :]    # [G, ktot] scaled
    Ws = W_cat * SW
    # fp8 blocks (i,f,o = first 768 cols of each quarter) in DoubleRow layout
    w8_host = np.empty((NQ, 128, KT8, 2, W8C), ml_dtypes.float8_e4m3)
    w16_host = np.empty((NQ, 128, KT16, DS), np.float16)
    for q in range(NQ):
        blk8 = q8(Ws[q * 1024:q * 1024 + W8C, :]).T         # [ktot, 768]
        w8_host[q] = blk8.reshape(KT8, 2, 128, W8C).transpose(2, 0, 1, 3)
        blk16 = Ws[q * 1024 + W8C:(q + 1) * 1024, :].T.astype(np.float16)
        w16_host[q] = blk16.reshape(KT16, 128, DS).transpose(1, 0, 2)

    bias_row = ((b_ih + b_hh)[idx] * (SX * SW)).astype(np.float32)
    bias = np.ascontiguousarray(np.broadcast_to(bias_row, (128, G)))

    xh = np.concatenate([input, h_prev], axis=1) * SX       # [B, ktot] scaled
    x8 = q8(xh)
    x16 = xh.astype(np.float16)
    c16 = c_prev.astype(np.float16)

    in_maps = []
    for c in range(n_cores):
        rows = slice(c * b_loc, (c + 1) * b_loc)
        xc8 = x8[rows].T                                    # [ktot, b_loc]
        xc16 = x16[rows].T
        # [p, b, t, s, m] = x[t*256 + s*128 + p, b*128 + m]
        xh8_h = xc8.reshape(KT8, 2, 128, BT, 128).transpose(2, 3, 0, 1, 4)
        xh16_h = xc16.reshape(KT16, 128, BT, 128).transpose(1, 2, 0, 3)
        in_maps.append({
            "xh8": np.ascontiguousarray(xh8_h),
            "xh16": np.ascontiguousarray(xh16_h),
            "w8": w8_host,
            "w16": w16_host,
            "bias": bias,
            "c_prev": np.ascontiguousarray(
                c16[rows].reshape(BT, 128, NQ, DS).transpose(1, 0, 2, 3)),
        })
    return in_maps


def run_lstm(inputs, trace=False, **spmd_kwargs):
    """Builds + runs the kernel on all 8 cores. Returns (h_t, c_t), results."""
    in_maps = prep_inputs(**inputs)
    nc = build_lstm_nc()
    res = run_bass_kernel_spmd(nc, in_maps, core_ids=list(range(N_CORES)),
                               trace=trace, **spmd_kwargs)
    ch = np.concatenate([r["ch_out"] for r in res.results], axis=0)
    c_t = ch[:, 0, :].astype(np.float32)
    h_t = ch[:, 1, :].astype(np.float32)
    return (h_t, c_t), res


def kernel(input, h_prev, c_prev, W_ih, b_ih, W_hh, b_hh):
    (h_t, c_t), _ = run_lstm(dict(
        input=input, h_prev=h_prev, c_prev=c_prev,
        W_ih=W_ih, b_ih=b_ih, W_hh=W_hh, b_hh=b_hh))
    return (h_t, c_t)
